# revision 4
# baseline (speedup 1.0000x reference)
"""Trainium2 Bass kernel for CosmicMultiHeadAttention (block-diagonal GQA).

Sharding: 8 cores = 2 (batch) x 4 (head groups). Each core handles one batch's
2048 tokens, 32 q heads, 4 kv heads. Wq/Wk/Wv column-sharded, Wo row-sharded;
host sums the 4 partial outputs per batch.

Pipeline per core (all matmuls on TensorE):
  Phase 1 (f32r): qT/kT/vT projections in head-transposed layout (c, t),
    contraction over E split in 2 halves with a DRAM partial roundtrip so the
    16 MB x-slab (E/2 x 1024 tokens) stays SBUF-resident while weights stream
    only twice. RoPE fused into the second half's epilogue on DVE.
  Phase 2: block-diagonal attention. Logits computed TRANSPOSED (k, q) so
    softmax normalization needs no A-transposes: exp on ACT -> bf16, row sums
    + partition broadcast via an all-ones bf16 matmul, A^T @ V via PE with V
    transposed on PE, normalization on DVE straight into a resident bf16
    attn-out tile.
  Phase 3 (bf16): out = attn_out @ Wo, attn-out stationary from SBUF, Wo
    streamed, fp32 psum DMA'd directly to DRAM.
"""

import os
import sys

for _p in ("/opt/trn_rl_repo", "/root/.axon_site/_ro/trn_rl_repo"):
    if os.path.isdir(_p) and _p not in sys.path:
        sys.path.insert(0, _p)

import numpy as np
import ml_dtypes

E = 8192
T = 2048          # tokens per core (one batch)
HQ, HKV, D, BS = 128, 16, 128, 128
CQ = 4096         # q columns per core (32 heads)
CKV = 512         # k/v columns per core (4 heads)
NCC = 40          # c-chunks per core: 32 q + 4 k + 4 v
EH = E // 2       # E half for the split-K projection
NB = T // BS      # 16 attention blocks
ROPE_BASE = 10000.0
SCALE = 1.0 / float(np.sqrt(np.float32(D)))


def round_f32r(a: np.ndarray) -> np.ndarray:
    """RNE to fp32r (fp32 keeping 11 mantissa bits / top 20 bits)."""
    u = np.ascontiguousarray(a, np.float32).view(np.uint32).astype(np.uint64)
    lsb = (u >> 12) & 1
    r = (u + 0x7FF + lsb) & np.uint64(0xFFFFF000)
    return r.astype(np.uint32).view(np.float32)


_MODULE_CACHE = {}


def build_module():
    if "nc" in _MODULE_CACHE:
        return _MODULE_CACHE["nc"]

    import concourse.bass as bass
    import concourse.tile as tile
    from concourse import mybir
    from concourse.bacc import Bacc
    from concourse.masks import make_identity

    F32R = mybir.dt.float32r
    BF16 = mybir.dt.bfloat16
    F32 = mybir.dt.float32

    nc = Bacc()

    xT = nc.declare_dram_parameter("xT", [E, T], F32R, isOutput=False)
    wq = nc.declare_dram_parameter("wq", [E, CQ], F32R, isOutput=False)
    wk = nc.declare_dram_parameter("wk", [E, CKV], F32R, isOutput=False)
    wv = nc.declare_dram_parameter("wv", [E, CKV], F32R, isOutput=False)
    wo = nc.declare_dram_parameter("wo", [CQ, E], BF16, isOutput=False)
    bias_col = nc.declare_dram_parameter("bias_col", [128, NCC], F32, isOutput=False)
    cosT = nc.declare_dram_parameter("cosT", [128, T], F32, isOutput=False)
    sinT = nc.declare_dram_parameter("sinT", [128, T], F32, isOutput=False)

    out_p = nc.declare_dram_parameter("out_p", [T, E], F32, isOutput=True)
    k_out = nc.declare_dram_parameter("k_out", [CKV, T], F32, isOutput=True)
    v_out = nc.declare_dram_parameter("v_out", [CKV, T], F32, isOutput=True)

    with tile.TileContext(nc) as tc:
        with tc.tile_pool(name="dram", bufs=1, space="DRAM") as dram:
            part_s = dram.tile([NCC * 128, T], F32)      # phase-1a partials
            qT_s = dram.tile([CQ, T], F32R)              # roped q, head-transposed
            vT_s = dram.tile([CKV, T], BF16)             # v for attention (bf16)

            # ---------------- Phase 1: QKV projections (f32r) ----------------
            with tc.tile_pool(name="p1_x", bufs=1) as p1_x, \
                 tc.tile_pool(name="p1_w", bufs=3) as p1_w, \
                 tc.tile_pool(name="p1_sb", bufs=2) as p1_sb, \
                 tc.tile_pool(name="p1_one", bufs=1) as p1_one, \
                 tc.tile_pool(name="p1_ps", bufs=2, space="PSUM") as p1_ps:

                bias_sb = p1_one.tile([128, NCC], F32)
                nc.sync.dma_start(out=bias_sb, in_=bias_col.ap())
                cos_sb = p1_one.tile([128, T], F32)
                sin_sb = p1_one.tile([128, T], F32)
                nc.sync.dma_start(out=cos_sb, in_=cosT.ap())
                nc.sync.dma_start(out=sin_sb, in_=sinT.ap())

                def w_src(cc):
                    # weight DRAM handle + column offset for chunk cc
                    if cc < 32:
                        return wq, cc * 128
                    if cc < 36:
                        return wk, (cc - 32) * 128
                    return wv, (cc - 36) * 128

                for half in (0, 1):
                    e0 = half * EH
                    for tt in (0, 1):
                        t0 = tt * 1024
                        xa = p1_x.tile([128, 32, 1024], F32R, name="xa")
                        xsrc = xT.ap()[e0:e0 + EH, t0:t0 + 1024] \
                            .rearrange("(ec p) t -> p ec t", p=128)
                        for ec in range(32):
                            nc.sync.dma_start(out=xa[:, ec, :], in_=xsrc[:, ec, :])
                        for cc in range(NCC):
                            wt, c0 = w_src(cc)
                            whs = []
                            for wh in (0, 1):
                                wpan = p1_w.tile([128, 16, 128], F32R, name="wpan")
                                nc.sync.dma_start(
                                    out=wpan,
                                    in_=wt.ap()[e0 + wh * 2048:e0 + (wh + 1) * 2048,
                                                c0:c0 + 128]
                                        .rearrange("(ec p) c -> p ec c", p=128),
                                )
                                whs.append(wpan)
                            psums = []
                            for th in (0, 1):
                                ps = p1_ps.tile([128, 512], F32, name=f"ps{th}",
                                                tag=f"ps{th}")
                                for ec in range(32):
                                    nc.tensor.matmul(
                                        ps,
                                        lhsT=whs[ec // 16][:, ec % 16, :],
                                        rhs=xa[:, ec, th * 512:(th + 1) * 512],
                                        start=(ec == 0),
                                        stop=(ec == 31),
                                    )
                                psums.append(ps)

                            pslice = part_s[cc * 128:(cc + 1) * 128, t0:t0 + 1024]
                            if half == 0:
                                stage = p1_sb.tile([128, 1024], F32, name="stage")
                                for th in (0, 1):
                                    nc.scalar.copy(
                                        out=stage[:, th * 512:(th + 1) * 512],
                                        in_=psums[th])
                                nc.sync.dma_start(out=pslice, in_=stage)
                                continue

                            # half == 1: combine + bias (+ RoPE for q/k) on DVE
                            part_sb = p1_sb.tile([128, 1024], F32, name="part_sb")
                            nc.sync.dma_start(out=part_sb, in_=pslice)
                            comb = p1_sb.tile([128, 1024], F32, name="comb", bufs=1)
                            for th in (0, 1):
                                sl = slice(th * 512, (th + 1) * 512)
                                nc.vector.scalar_tensor_tensor(
                                    out=comb[:, sl],
                                    in0=psums[th],
                                    scalar=bias_sb[:, cc:cc + 1],
                                    in1=part_sb[:, sl],
                                    op0=mybir.AluOpType.add,
                                    op1=mybir.AluOpType.add,
                                )

                            if cc < 36:
                                # RoPE: out = comb*cos + rot(comb)*sin_signed
                                rot = p1_sb.tile([128, 1024], F32, name="rot", bufs=1)
                                nc.vector.tensor_copy(rot[0:64, :], comb[64:128, :])
                                nc.vector.tensor_copy(rot[64:128, :], comb[0:64, :])
                                nc.vector.tensor_mul(
                                    rot, rot, sin_sb[:, t0:t0 + 1024])
                                nc.vector.tensor_mul(
                                    comb, comb, cos_sb[:, t0:t0 + 1024])
                                roped = p1_sb.tile([128, 1024], F32R, name="roped")
                                nc.vector.tensor_add(roped, comb, rot)
                                if cc < 32:
                                    nc.sync.dma_start(
                                        out=qT_s[cc * 128:(cc + 1) * 128,
                                                 t0:t0 + 1024],
                                        in_=roped)
                                else:
                                    r = cc - 32
                                    nc.sync.dma_start(
                                        out=k_out.ap()
                                            .bitcast(F32R)[r * 128:(r + 1) * 128,
                                                           t0:t0 + 1024],
                                        in_=roped)
                            else:
                                r = cc - 36
                                nc.sync.dma_start(
                                    out=v_out.ap()[r * 128:(r + 1) * 128,
                                                   t0:t0 + 1024],
                                    in_=comb)
                                vbf = p1_sb.tile([128, 1024], BF16, name="vbf")
                                nc.vector.tensor_copy(vbf, comb)
                                nc.sync.dma_start(
                                    out=vT_s[r * 128:(r + 1) * 128, t0:t0 + 1024],
                                    in_=vbf)

            # -------- Phase 2: block attention + Phase 3: output matmul --------
            with tc.tile_pool(name="ao", bufs=1) as ao_pool, \
                 tc.tile_pool(name="p2_one", bufs=1) as p2_one, \
                 tc.tile_pool(name="p2_sb", bufs=2) as p2_sb, \
                 tc.tile_pool(name="p2_ps", bufs=1, space="PSUM") as p2_ps, \
                 tc.tile_pool(name="p3_w", bufs=2) as p3_w, \
                 tc.tile_pool(name="p3_ps", bufs=2, space="PSUM") as p3_ps:

                import concourse.mybir as _mb
                aoT = ao_pool.tile([128, 32, T], BF16)

                ones_bf = p2_one.tile([128, 128], BF16)
                nc.vector.memset(ones_bf, 1.0)
                ident_bf = p2_one.tile([128, 128], BF16)
                make_identity(nc, ident_bf)

                qT_r = qT_s.rearrange("(c p) t -> p c t", p=128)
                kT_r = k_out.ap().bitcast(F32R).rearrange("(c p) t -> p c t", p=128)
                vT_r = vT_s.rearrange("(c p) t -> p c t", p=128)

                for n in range(NB):
                    ts = slice(n * 128, (n + 1) * 128)
                    for h in range(4):
                        # v block: transpose (d,k)->(k,d) on PE once per (n,h)
                        vt_sb = p2_sb.tile([128, 128], BF16, name="vt_sb")
                        nc.sync.dma_start(out=vt_sb, in_=vT_r[:, h, ts])
                        vn_ps = p2_ps.tile([128, 128], BF16, name="vn_ps",
                                           tag="vn_ps")
                        nc.tensor.transpose(vn_ps, vt_sb, ident_bf)
                        vnat = p2_sb.tile([128, 128], BF16, name="vnat")
                        nc.vector.tensor_copy(vnat, vn_ps)

                        kt_sb = p2_sb.tile([128, 128], F32R, name="kt_sb")
                        nc.sync.dma_start(out=kt_sb, in_=kT_r[:, h, ts])

                        for u in (0, 1):
                            cc0 = h * 8 + u * 4
                            q4 = p2_sb.tile([128, 4, 128], F32R, name="q4")
                            nc.sync.dma_start(
                                out=q4, in_=qT_r[:, cc0:cc0 + 4, ts])
                            lt_ps = p2_ps.tile([128, 512], F32, name="lt_ps",
                                               tag="lt_ps", bufs=2)
                            nc.tensor.matmul(lt_ps, lhsT=kt_sb,
                                             rhs=q4.rearrange("p g t -> p (g t)"),
                                             start=True, stop=True)
                            at = p2_sb.tile([128, 512], BF16, name="at")
                            nc.scalar.activation(
                                out=at, in_=lt_ps,
                                func=_mb.ActivationFunctionType.Exp, scale=SCALE)
                            sum_ps = p2_ps.tile([128, 512], F32, name="sum_ps",
                                                tag="sum_ps")
                            nc.tensor.matmul(sum_ps, lhsT=ones_bf, rhs=at,
                                             start=True, stop=True)
                            ot_ps = p2_ps.tile([128, 512], F32, name="ot_ps",
                                               tag="ot_ps", bufs=2)
                            nc.tensor.matmul(ot_ps, lhsT=vnat, rhs=at,
                                             start=True, stop=True)
                            rbc = p2_sb.tile([128, 512], F32, name="rbc")
                            nc.vector.reciprocal(rbc, sum_ps)
                            nc.vector.tensor_mul(
                                aoT[:, cc0:cc0 + 4, ts],
                                ot_ps.rearrange("p (g t) -> p g t", t=128),
                                rbc.rearrange("p (g t) -> p g t", t=128))

                # Phase 3: out[t, e] += aoT[c, t] * wo[c, e]  (bf16)
                wo_r = wo.ap().rearrange("(cc p) e -> p cc e", p=128)
                for et in range(16):
                    es = slice(et * 512, (et + 1) * 512)
                    slabs = []
                    for whf in (0, 1):
                        slab = p3_w.tile([128, 16, 512], BF16, name="slab")
                        nc.sync.dma_start(
                            out=slab, in_=wo_r[:, whf * 16:(whf + 1) * 16, es])
                        slabs.append(slab)
                    for tb in range(16):
                        ps_o = p3_ps.tile([128, 512], F32, name="ps_o")
                        for cc in range(32):
                            nc.tensor.matmul(
                                ps_o,
                                lhsT=aoT[:, cc, tb * 128:(tb + 1) * 128],
                                rhs=slabs[cc // 16][:, cc % 16, :],
                                start=(cc == 0),
                                stop=(cc == 31),
                            )
                        o_sb = p3_w.tile([128, 512], F32, name="o_sb", bufs=3)
                        nc.scalar.copy(out=o_sb, in_=ps_o)
                        nc.sync.dma_start(
                            out=out_p.ap()[tb * 128:(tb + 1) * 128, es],
                            in_=o_sb)

    nc.compile()
    _MODULE_CACHE["nc"] = nc
    return nc


def _prep_inputs(x, Wq, bq, Wk, bk, Wv, bv, Wo, bo):
    """Build the 8 per-core input maps."""
    inv_freq = 1.0 / (ROPE_BASE ** (np.arange(0, D, 2, dtype=np.float32) / D))
    pos = np.arange(T, dtype=np.float32)
    ang = pos[None, :] * inv_freq[:, None]            # (64, T)
    cos_h = np.cos(ang).astype(np.float32)
    sin_h = np.sin(ang).astype(np.float32)
    cosT = np.concatenate([cos_h, cos_h], axis=0)     # (128, T)
    sinT = np.concatenate([-sin_h, sin_h], axis=0)    # signed for rot trick

    xr = [round_f32r(np.ascontiguousarray(np.asarray(x)[b].T)) for b in range(2)]
    Wq_r = round_f32r(np.asarray(Wq))
    Wk_r = round_f32r(np.asarray(Wk))
    Wv_r = round_f32r(np.asarray(Wv))
    Wo_bf = np.asarray(Wo).astype(ml_dtypes.bfloat16)

    bq = np.asarray(bq, np.float32)
    bk = np.asarray(bk, np.float32)
    bv = np.asarray(bv, np.float32)

    in_maps = []
    for core in range(8):
        b, hg = core // 4, core % 4
        bias_col = np.zeros((128, NCC), np.float32)
        for cc in range(32):
            bias_col[:, cc] = bq[hg * CQ + cc * 128:(hg * CQ) + (cc + 1) * 128]
        for r in range(4):
            bias_col[:, 32 + r] = bk[hg * CKV + r * 128:hg * CKV + (r + 1) * 128]
            bias_col[:, 36 + r] = bv[hg * CKV + r * 128:hg * CKV + (r + 1) * 128]
        in_maps.append({
            "xT": xr[b],
            "wq": np.ascontiguousarray(Wq_r[:, hg * CQ:(hg + 1) * CQ]),
            "wk": np.ascontiguousarray(Wk_r[:, hg * CKV:(hg + 1) * CKV]),
            "wv": np.ascontiguousarray(Wv_r[:, hg * CKV:(hg + 1) * CKV]),
            "wo": np.ascontiguousarray(Wo_bf[hg * CQ:(hg + 1) * CQ, :]),
            "bias_col": bias_col,
            "cosT": cosT,
            "sinT": sinT,
        })
    return in_maps


def _assemble(results, bo):
    bo = np.asarray(bo, np.float32)
    out = np.empty((2, T, E), np.float32)
    k = np.empty((2, T, HKV, D), np.float32)
    v = np.empty((2, T, HKV, D), np.float32)
    for b in range(2):
        acc = None
        for hg in range(4):
            r = results[b * 4 + hg]
            acc = r["out_p"] if acc is None else acc + r["out_p"]
            kk = r["k_out"].reshape(4, 128, T)        # (h, d, t)
            vv = r["v_out"].reshape(4, 128, T)
            k[b, :, hg * 4:(hg + 1) * 4, :] = kk.transpose(2, 0, 1)
            v[b, :, hg * 4:(hg + 1) * 4, :] = vv.transpose(2, 0, 1)
        out[b] = acc + bo
    return out, k, v


def kernel(x, Wq, bq, Wk, bk, Wv, bv, Wo, bo, _trace=False):
    from concourse.bass_utils import run_bass_kernel_spmd

    nc = build_module()
    in_maps = _prep_inputs(x, Wq, bq, Wk, bk, Wv, bv, Wo, bo)
    res = run_bass_kernel_spmd(nc, in_maps, core_ids=list(range(8)),
                               trace=_trace)
    out, k, v = _assemble(res.results, bo)
    if _trace:
        kernel.last_exec_time_ns = res.exec_time_ns
        kernel.last_trace = res.instructions_and_trace
    return out, k, v
